# revision 23
# baseline (speedup 1.0000x reference)
"""BiTreeLSTM Trainium2 kernel (8 NeuronCores, SPMD, batch-sharded).

Strategy
--------
Host (numpy): compute per-tree node depths from `parents`; process the
recurrence LEVEL-synchronously (all nodes at one depth are independent)
instead of the reference's 256 sequential steps.  Each core owns 16
trees and runs both directions (dt bottom-up, td top-down) as two
phases of one SPMD program.  Within a level, nodes are ordered grouped
by parent (parents in the previous level's order), which makes both
the td parent-gather and the dt children-segment-sum a STAIRCASE 0/1
matrix; gathers become small banded matmuls.

Device (Bass/Tile): per level
  sumHT  = prevH^T-gather via matmul (feature-major, feeds gates lhsT)
  sumC   = prevC row-major gather via matmul
  gates  = X^T @ W4x + sumH^T @ W4h   (W4 = [i,o,f,u] fused, 4H=1024)
  i,o,f  = sigmoid(gates[0:768]); u = tanh(gates[768:1024])   (ACT)
  c      = i*u + f*sumC;  h = o*tanh(c)                        (DVE)
All matmuls run in float32r (full fp32 storage, reduced-precision PE
multiply, 1 cycle/row).  PSUM accumulation groups with partial column
coverage are opened by a "zeroing matmul" (zeros lhsT, full width,
start=True) so untouched elements are exact zeros.

Schedules are data-dependent: level sizes are padded to the max over
the 8 cores (rounded to 128) so a single program serves all cores.
Host post-pass unpermutes the level-ordered outputs into [L, B, 2H].
"""

import os
import numpy as np

L, B, D, H = 256, 128, 256, 256
NCORES = 8
TPC = B // NCORES  # trees per core
H4 = 4 * H  # fused gate width (i,o,f,u)

_TRACE = os.environ.get("BITREE_TRACE", "0") == "1"
LAST_EXEC_NS = None

_CACHE = {}


def _install_ntff_shim():
    """Register the NTFF profile hook so trace=True works under axon."""
    import sys
    import types

    if "antenv.axon_hooks" in sys.modules:
        return
    hook_box = [None]
    mod = types.ModuleType("antenv.axon_hooks")
    mod.set_axon_ntff_profile_hook = lambda h: hook_box.__setitem__(0, h)
    mod.get_axon_ntff_profile_hook = lambda: hook_box[0]
    import antenv

    antenv.axon_hooks = mod
    sys.modules["antenv.axon_hooks"] = mod
    from trn_agent_boot.trn_boot import _ntff_profile_via_ctypes

    hook = _ntff_profile_via_ctypes("/opt/axon/libaxon_pjrt.so")
    if hook is not None:
        mod.set_axon_ntff_profile_hook(hook)


def _ceil128(x):
    return ((int(x) + 127) // 128) * 128


def _balance_trees(depth, NLV):
    """Assign trees to cores minimizing sum_d ceil128(max_core level_size)."""
    nd = np.zeros((NLV, B), np.int64)
    for b in range(B):
        cnt = np.bincount(depth[:, b], minlength=NLV)
        nd[:, b] = cnt
    def cost_of(p):
        # primary: padded chunk count; secondary: raw imbalance (plateau guide)
        return (int(((np.max(p, axis=1) + 127) // 128).sum()),
                int(np.max(p, axis=1).sum()))

    best_cost, best_cores = None, None
    for seed in range(3):
        rng = np.random.default_rng(seed)
        peak = nd.max(axis=0)
        orderb = np.argsort(-peak, kind="stable")
        cores = [[] for _ in range(NCORES)]
        prof = np.zeros((NLV, NCORES), np.int64)
        for b in orderb:
            bestj, bi = None, -1
            for c in range(NCORES):
                if len(cores[c]) >= TPC:
                    continue
                p = prof.copy()
                p[:, c] += nd[:, b]
                j = cost_of(p) + (int(p[:, c].sum()),)
                if bestj is None or j < bestj:
                    bestj, bi = j, c
            cores[bi].append(int(b))
            prof[:, bi] += nd[:, b]
        cur = cost_of(prof)
        for _ in range(20000):
            c1, c2 = rng.integers(0, NCORES, 2)
            if c1 == c2:
                continue
            i1, i2 = rng.integers(0, TPC, 2)
            b1, b2 = cores[c1][i1], cores[c2][i2]
            p = prof.copy()
            p[:, c1] += nd[:, b2] - nd[:, b1]
            p[:, c2] += nd[:, b1] - nd[:, b2]
            j = cost_of(p)
            if j <= cur:
                cur = j
                prof = p
                cores[c1][i1], cores[c2][i2] = b2, b1
        if best_cost is None or cur < best_cost:
            best_cost, best_cores = cur, [list(c) for c in cores]
    cores = best_cores
    tree_core = np.zeros(B, np.int64)
    for c in range(NCORES):
        cores[c].sort()
        for b in cores[c]:
            tree_core[b] = c
    return tree_core, [list(c) for c in cores]


def _build_schedule(parents):
    """Level schedule + gather-matrix band metadata, uniform across cores."""
    par = np.asarray(parents, dtype=np.int64)  # [L, B], par[0,:]=L
    depth = np.zeros((L, B), np.int64)
    bar = np.arange(B)
    for i in range(1, L):
        depth[i] = depth[par[i], bar] + 1
    DMAX = int(depth.max())
    NLV = DMAX + 1
    tree_core, core_trees = _balance_trees(depth, NLV)

    # per (level, core): ordered list of (tree, node); parent-grouped order
    order = [[[] for _ in range(NCORES)] for _ in range(NLV)]
    pos = np.full((L, B), -1, np.int64)
    for b in range(B):
        core = int(tree_core[b])
        kids = [[] for _ in range(L)]
        for i in range(1, L):
            kids[par[i, b]].append(i)
        cur = [0]
        d = 0
        while cur:
            od = order[d][core]
            for n in cur:
                pos[n, b] = len(od)
                od.append((b, n))
            nxt = []
            for n in cur:
                nxt.extend(kids[n])
            cur = nxt
            d += 1

    n_real = np.zeros((NLV, NCORES), np.int64)
    for d in range(NLV):
        for c in range(NCORES):
            n_real[d, c] = len(order[d][c])
    NPAD = [_ceil128(n_real[d].max()) for d in range(NLV)]

    # processing sequence: (phase, level, prev_level or None)
    # dt and td are independent chains — interleave them step-wise so each
    # chain's level-boundary dependency stall is covered by the other's work.
    dt_steps = [("dt", d, d + 1 if d < DMAX else None)
                for d in range(DMAX, -1, -1)]
    td_steps = [("td", d, d - 1 if d > 0 else None)
                for d in range(0, DMAX + 1)]
    steps = []
    lead = 2
    steps.extend(dt_steps[:lead])
    for k, b_ in enumerate(td_steps):
        steps.append(b_)
        if lead + k < len(dt_steps):
            steps.append(dt_steps[lead + k])

    # gather matrices: for step (phase, dl, pl): GT [m=NPAD[pl], n=NPAD[dl]]
    #   dt: GT[j, r] = 1 iff parent(order[pl][j]) == order[dl][r]
    #   td: GT[p, j] = 1 iff parent(order[dl][j]) == order[pl][p]
    # Build per-core col indices once (parent positions).
    def _gt_entries(phase, dl, pl, core):
        """row_idx, col_idx arrays of the 1-entries for this core."""
        if phase == "dt":
            ent = order[pl][core]  # children level
            rows = np.arange(len(ent), dtype=np.int64)
            cols = np.array([pos[par[n, b], b] for (b, n) in ent], dtype=np.int64)
        else:
            ent = order[dl][core]
            cols = np.arange(len(ent), dtype=np.int64)
            rows = np.array([pos[par[n, b], b] for (b, n) in ent], dtype=np.int64)
        return rows, cols

    # Band metadata per gather step: used chunks, 128-aligned col spans,
    # per-128-col-block windows (win1) and per-512-col-block windows (win2).
    gmeta = {}  # (phase, dl) -> dict
    for phase, dl, pl in steps:
        if pl is None:
            continue
        m, n = NPAD[pl], NPAD[dl]
        mch = m // 128
        # per-chunk tight col ranges, unioned over cores
        clo = np.full(mch, n, np.int64)
        chi = np.full(mch, -1, np.int64)
        ents = []
        for core in range(NCORES):
            rows, cols = _gt_entries(phase, dl, pl, core)
            ents.append((rows, cols))
            if len(rows):
                ch = rows // 128
                np.minimum.at(clo, ch, cols)
                np.maximum.at(chi, ch, cols)
        used = [c for c in range(mch) if chi[c] >= 0]
        span = {}
        for c in used:
            a = 128 * (clo[c] // 128)
            bnd = min(n, _ceil128(chi[c] + 1))
            span[c] = (int(a), int(bnd))
        # tight per-(chunk, 512-col-block) column ranges for gather2 matmuls
        nbk = (n + 511) // 512
        cliplo = np.full((mch, nbk), n, np.int64)
        cliphi = np.full((mch, nbk), -1, np.int64)
        for rows, cols in ents:
            if len(rows):
                key = (rows // 128, cols // 512)
                np.minimum.at(cliplo, key, cols)
                np.maximum.at(cliphi, key, cols + 1)
        clip = {(int(c), int(nb)): (int(cliplo[c, nb]) // 2 * 2,
                                    min(n, 512 * int(nb) + 512,
                                        (int(cliphi[c, nb]) + 1) // 2 * 2))
                for c in range(mch) for nb in range(nbk) if cliphi[c, nb] >= 0}
        # win1: per 128-col block i -> list of chunks with a 1 in that block
        # win2: per 512-col block nb -> same
        nch = n // 128
        w1 = [[] for _ in range(nch)]
        nb_cnt = (n + 511) // 512
        w2 = [[] for _ in range(nb_cnt)]
        touch1 = np.zeros((mch, nch), bool)
        for rows, cols in ents:
            if len(rows):
                touch1[rows // 128, cols // 128] = True
        for c in used:
            for i in range(nch):
                if touch1[c, i]:
                    w1[i].append(c)
            for nb in range(nb_cnt):
                if touch1[c, 4 * nb : min(nch, 4 * nb + 4)].any():
                    w2[nb].append(c)
        gmeta[(phase, dl)] = dict(used=used, span=span, w1=w1, w2=w2, m=m, n=n, clip=clip)
        # store entries for data build
        gmeta[(phase, dl)]["ents"] = ents

    # layout offsets
    xt_off, acc = [], 0
    for d in range(NLV):
        xt_off.append(acc)
        acc += 2 * NPAD[d]
    XTW = acc

    gt_off = {}  # (phase, dl, chunk) -> col offset in gt tensor
    acc = 0
    gt_level_off = {}
    for phase, dl, pl in steps:
        if pl is None:
            continue
        gm = gmeta[(phase, dl)]
        gt_level_off[(phase, dl)] = acc
        for c in gm["used"]:
            a, bnd = gm["span"][c]
            gt_off[(phase, dl, c)] = acc
            acc += bnd - a
    GTW = max(acc, 128)

    out_off = {}
    acc = 0
    for phase, dl, pl in steps:
        out_off[(phase, dl)] = acc
        acc += NPAD[dl]
    OUTR = acc

    return dict(
        DMAX=DMAX, NLV=NLV, order=order, pos=pos, n_real=n_real, NPAD=NPAD,
        steps=steps, gmeta=gmeta, xt_off=xt_off, XTW=XTW, gt_off=gt_off,
        gt_level_off=gt_level_off, GTW=GTW, out_off=out_off, OUTR=OUTR,
        tree_core=tree_core,
    )


def _build_core_inputs(sched, inputs_np, weights):
    """Per-core numpy arrays: xt [128, XTW], gt [128, GTW], shared w4t."""
    NPAD, xt_off, XTW = sched["NPAD"], sched["xt_off"], sched["XTW"]
    GTW, gt_off = sched["GTW"], sched["gt_off"]
    order, steps, gmeta = sched["order"], sched["steps"], sched["gmeta"]
    NLV = sched["NLV"]

    xts, gts = [], []
    for core in range(NCORES):
        xt = np.zeros((128, XTW), np.float32)
        for d in range(NLV):
            ent = order[d][core]
            if ent:
                bs = np.array([b for b, n in ent])
                ns = np.array([n for b, n in ent])
                xl = inputs_np[ns, bs, :]  # [n_d, 256]
                xlT = xl.T  # [256, n_d]
                o = xt_off[d]
                xt[:, o : o + len(ent)] = xlT[:128]
                xt[:, o + NPAD[d] : o + NPAD[d] + len(ent)] = xlT[128:]
        xts.append(xt)

        gt = np.zeros((128, GTW), np.float32)
        for phase, dl, pl in steps:
            if pl is None:
                continue
            gm = gmeta[(phase, dl)]
            rows, cols = gm["ents"][core]
            if not len(rows):
                continue
            for c in gm["used"]:
                a, bnd = gm["span"][c]
                msk = (rows // 128) == c
                if not msk.any():
                    continue
                r = rows[msk] - 128 * c
                cc = cols[msk] - a
                o = gt_off[(phase, dl, c)]
                gt[r, o + cc] = 1.0
        gts.append(gt)

    # fused weights: per direction, rows ordered [i, o, f, u], transposed.
    # layout [128, 8192]: dir (dt=0, td=1) at 4096*dir; x-part chunks k at
    # [doff + k*1024, +1024), h-part at [doff + 2048 + k*1024, +1024).
    w4t = np.zeros((128, 8192), np.float32)
    for di, pre in enumerate(("dt", "td")):
        ioux, iouh = weights[f"{pre}_ioux_w"], weights[f"{pre}_iouh_w"]
        fx, fh = weights[f"{pre}_fx_w"], weights[f"{pre}_fh_w"]
        wx = np.concatenate([ioux[0:256], fx, ioux[256:512], ioux[512:768]], 0)
        wh = np.concatenate([iouh[0:256], fh, iouh[256:512], iouh[512:768]], 0)
        for k in range(2):
            w4t[:, di * 4096 + k * 1024 : di * 4096 + (k + 1) * 1024] = \
                wx.T[k * 128 : (k + 1) * 128]
            w4t[:, di * 4096 + 2048 + k * 1024 : di * 4096 + 2048 + (k + 1) * 1024] = \
                wh.T[k * 128 : (k + 1) * 128]
    return xts, gts, w4t


def _build_program(sched):
    from contextlib import ExitStack

    import concourse.tile as tile
    from concourse import bacc, mybir

    f32 = mybir.dt.float32
    f32r = mybir.dt.float32r
    SIG = mybir.ActivationFunctionType.Sigmoid
    TANH = mybir.ActivationFunctionType.Tanh

    NPAD, xt_off = sched["NPAD"], sched["xt_off"]
    XTW, GTW, OUTR = sched["XTW"], sched["GTW"], sched["OUTR"]
    steps, gmeta = sched["steps"], sched["gmeta"]
    gt_off, gt_level_off = sched["gt_off"], sched["gt_level_off"]
    out_off = sched["out_off"]

    nc = bacc.Bacc("TRN2", target_bir_lowering=False, debug=False,
                   num_devices=NCORES)

    xt_ap = nc.dram_tensor("xt", [128, XTW], f32r, kind="ExternalInput").ap()
    gt_ap = nc.dram_tensor("gt", [128, GTW], f32r, kind="ExternalInput").ap()
    w4_ap = nc.dram_tensor("w4t", [128, 8192], f32r, kind="ExternalInput").ap()
    z_ap = nc.dram_tensor("zeros", [128, 128], f32r, kind="ExternalInput").ap()
    oc_ap = nc.dram_tensor("out_c", [OUTR, 256], f32, kind="ExternalOutput").ap()
    oh_ap = nc.dram_tensor("out_h", [OUTR, 256], f32, kind="ExternalOutput").ap()

    with tile.TileContext(nc) as tc:
        with ExitStack() as ctx:
            const = ctx.enter_context(tc.tile_pool(name="const", bufs=1))
            xt_pool = ctx.enter_context(tc.tile_pool(name="xt", bufs=3))
            gt_pool = ctx.enter_context(tc.tile_pool(name="gt", bufs=3))
            cpool = ctx.enter_context(tc.tile_pool(name="stc", bufs=2))
            hpool = ctx.enter_context(tc.tile_pool(name="sth", bufs=2))
            sht_pool = ctx.enter_context(tc.tile_pool(name="sht", bufs=3))
            tmp = ctx.enter_context(tc.tile_pool(name="tmp", bufs=3))
            ps2s = ctx.enter_context(tc.tile_pool(name="ps2", bufs=2, space="PSUM"))
            ps2 = {"dt": ps2s, "td": ps2s}
            ps1s = ctx.enter_context(tc.tile_pool(name="ps1", bufs=2, space="PSUM"))
            ps1 = {"dt": ps1s, "td": ps1s}
            psg = ctx.enter_context(tc.tile_pool(name="psg", bufs=2, space="PSUM"))

            # weights as 4 lazily-loaded tiles (dir x {x-part, h-part}) so the
            # first gate matmul only waits on its own 1MB slice.
            w4_tiles = {}

            def w4_tile(di, part):  # part: 0=x, 1=h
                key = (di, part)
                if key not in w4_tiles:
                    t = const.tile([128, 2048], f32r, name=f"w4_{di}_{part}")
                    nc.sync.dma_start(
                        t[:], w4_ap[:, di * 4096 + part * 2048 :
                                    di * 4096 + part * 2048 + 2048])
                    w4_tiles[key] = t
                return w4_tiles[key]

            zer_t = const.tile([128, 128], f32r)
            nc.sync.dma_start(zer_t[:], z_ap[:])

            prev = {"dt": (None, None), "td": (None, None)}
            for phase, dl, pl in steps:
                di = 0 if phase == "dt" else 1
                prev_c, prev_h = prev[phase]
                n = NPAD[dl]
                nch = n // 128
                xo = xt_off[dl]

                xt_t = xt_pool.tile([128, 2 * n], f32r, tag="xt")
                nc.sync.dma_start(xt_t[:], xt_ap[:, xo : xo + 2 * n])

                cur_c = cpool.tile([128, nch * 256], f32r, tag=f"stc_{phase}")
                cur_h = hpool.tile([128, nch * 256], f32r, tag=f"sth_{phase}")

                has_prev = pl is not None
                if has_prev:
                    gm = gmeta[(phase, dl)]
                    lvl_go = gt_level_off[(phase, dl)]
                    lvl_w = sum(gm["span"][c][1] - gm["span"][c][0]
                                for c in gm["used"])
                    gt_t = gt_pool.tile([128, max(lvl_w, 128)], f32r, tag="gt")
                    if lvl_w:
                        nc.sync.dma_start(gt_t[:, :lvl_w],
                                          gt_ap[:, lvl_go : lvl_go + lvl_w])

                    # gather2: sumHT [2][128, n] feature-major
                    sht = sht_pool.tile([128, 2 * n], f32r, tag="sht")
                    for j in range(2):
                        for nb in range((n + 511) // 512):
                            nb0 = 512 * nb
                            wb = min(512, n - nb0)
                            mms = []
                            for c in gm["w2"][nb]:
                                a, bnd = gm["span"][c]
                                lo, hi = gm["clip"][(c, nb)]
                                if lo < hi:
                                    mms.append((c, a, lo, hi))
                            pst = ps2[phase].tile([128, wb], f32, tag="ps2")
                            nc.tensor.matmul(pst[:], zer_t[:],
                                             xt_t[:, 0:wb],
                                             start=True, stop=not mms,
                                             skip_group_check=True)
                            for mi, (c, a, lo, hi) in enumerate(mms):
                                go = gt_off[(phase, dl, c)] - lvl_go
                                nc.tensor.matmul(
                                    pst[:, lo - nb0 : hi - nb0],
                                    prev_h[:, c * 256 + j * 128 :
                                           c * 256 + j * 128 + 128],
                                    gt_t[:, go + lo - a : go + hi - a],
                                    start=False, stop=(mi == len(mms) - 1),
                                    skip_group_check=True)
                            nc.vector.tensor_copy(
                                sht[:, j * n + nb0 : j * n + nb0 + wb], pst[:])

                for i in range(nch):
                    io = i * 128
                    # gather1: sumC for this row-chunk
                    ps_c = None
                    if has_prev:
                        gm = gmeta[(phase, dl)]
                        lvl_go = gt_level_off[(phase, dl)]
                        ps_c = ps1[phase].tile([128, 256], f32, tag="ps1")
                        w1 = gm["w1"][i]
                        if not w1:
                            nc.tensor.matmul(ps_c[:], zer_t[:],
                                             prev_c[:, 0:256],
                                             start=True, stop=True,
                                             skip_group_check=True)
                        # every real matmul fully covers [128, 256] (spans are
                        # 128-aligned), so the first one opens the group.
                        for ci, c in enumerate(w1):
                            a, bnd = gm["span"][c]
                            go = gt_off[(phase, dl, c)] - lvl_go
                            nc.tensor.matmul(
                                ps_c[:], gt_t[:, go + io - a : go + io - a + 128],
                                prev_c[:, c * 256 : c * 256 + 256],
                                start=(ci == 0), stop=(ci == len(w1) - 1),
                                skip_group_check=True)

                    # gates: [128, 1024] = X^T W4x (+ sumH^T W4h)
                    w4x = w4_tile(di, 0)
                    ps_g = psg.tile([128, 1024], f32, tag="psg")
                    for half in range(2):
                        ho = half * 512
                        for k in range(2):
                            nc.tensor.matmul(
                                ps_g[:, ho : ho + 512],
                                xt_t[:, k * n + io : k * n + io + 128],
                                w4x[:, k * 1024 + ho : k * 1024 + ho + 512],
                                start=(k == 0), stop=(k == 1 and not has_prev),
                                skip_group_check=True)
                        if has_prev:
                            w4h = w4_tile(di, 1)
                            for k in range(2):
                                nc.tensor.matmul(
                                    ps_g[:, ho : ho + 512],
                                    sht[:, k * n + io : k * n + io + 128],
                                    w4h[:, k * 1024 + ho : k * 1024 + ho + 512],
                                    start=False, stop=(k == 1),
                                    skip_group_check=True)

                    # cell math (gate order [i, f, o, u]; o off the c-path)
                    s_if = tmp.tile([128, 512], f32, tag="sif")
                    nc.scalar.activation(s_if[:], ps_g[:, 0:512], SIG)
                    u_t = tmp.tile([128, 256], f32, tag="ut")
                    nc.scalar.activation(u_t[:], ps_g[:, 768:1024], TANH)

                    ccol = cur_c[:, i * 256 : i * 256 + 256]
                    hcol = cur_h[:, i * 256 : i * 256 + 256]
                    if has_prev:
                        t1 = tmp.tile([128, 256], f32, tag="t1")
                        nc.vector.tensor_mul(t1[:], s_if[:, 0:256], u_t[:])
                        t2 = tmp.tile([128, 256], f32, tag="t2")
                        nc.vector.tensor_mul(t2[:], s_if[:, 256:512], ps_c[:])
                        nc.vector.tensor_add(ccol, t1[:], t2[:])
                    else:
                        nc.vector.tensor_mul(ccol, s_if[:, 0:256], u_t[:])
                    tc_t = tmp.tile([128, 256], f32, tag="tct")
                    nc.scalar.activation(tc_t[:], ccol.bitcast(f32), TANH)
                    s_o = tmp.tile([128, 256], f32, tag="so")
                    nc.scalar.activation(s_o[:], ps_g[:, 512:768], SIG)
                    nc.vector.tensor_mul(hcol, s_o[:], tc_t[:])

                ro = out_off[(phase, dl)]
                oc_v = oc_ap[ro : ro + n, :].rearrange(
                    "(c p) f -> p c f", p=128)
                oh_v = oh_ap[ro : ro + n, :].rearrange(
                    "(c p) f -> p c f", p=128)
                nc.sync.dma_start(
                    oc_v, cur_c[:].bitcast(f32).rearrange("p (c f) -> p c f", f=256))
                nc.sync.dma_start(
                    oh_v, cur_h[:].bitcast(f32).rearrange("p (c f) -> p c f", f=256))

                prev[phase] = (cur_c, cur_h)

    nc.compile()
    return nc


def kernel(**inputs):
    global LAST_EXEC_NS
    inp = {k: np.asarray(v) for k, v in inputs.items()}
    x = inp["inputs"].astype(np.float32)
    parents = inp["parents"]

    for pre in ("dt", "td"):
        for nm in ("ioux", "iouh", "fx", "fh"):
            if np.any(inp[f"{pre}_{nm}_b"] != 0):
                raise NotImplementedError("nonzero biases not supported")

    key = parents.tobytes()
    if key not in _CACHE:
        sched = _build_schedule(parents)
        prog = _build_program(sched)
        _CACHE[key] = (sched, prog)
    sched, prog = _CACHE[key]

    xts, gts, w4t = _build_core_inputs(sched, x, inp)
    zeros = np.zeros((128, 128), np.float32)
    in_maps = [
        {"xt": xts[c], "gt": gts[c], "w4t": w4t, "zeros": zeros}
        for c in range(NCORES)
    ]

    from concourse.bass_utils import run_bass_kernel_spmd

    if _TRACE:
        _install_ntff_shim()
        res = run_bass_kernel_spmd(prog, in_maps, list(range(NCORES)), trace=True)
        LAST_EXEC_NS = res.exec_time_ns
        print(f"HW exec time: {res.exec_time_ns} ns")
    else:
        res = run_bass_kernel_spmd(prog, in_maps, list(range(NCORES)))

    cells = np.zeros((L, B, 2 * H), np.float32)
    hiddens = np.zeros((L, B, 2 * H), np.float32)
    order, n_real = sched["order"], sched["n_real"]
    out_off, NLV = sched["out_off"], sched["NLV"]
    for core in range(NCORES):
        oc = res.results[core]["out_c"]
        oh = res.results[core]["out_h"]
        for phase, sl in (("dt", slice(0, H)), ("td", slice(H, 2 * H))):
            for d in range(NLV):
                ent = order[d][core]
                if not ent:
                    continue
                o = out_off[(phase, d)]
                bs = np.array([b for b, n_ in ent])
                ns = np.array([n_ for b, n_ in ent])
                cells[ns, bs, sl] = oc[o : o + len(ent)]
                hiddens[ns, bs, sl] = oh[o : o + len(ent)]
    return cells, hiddens


# revision 24
# speedup vs baseline: 1.0148x; 1.0148x over previous
"""BiTreeLSTM Trainium2 kernel (8 NeuronCores, SPMD, batch-sharded).

Strategy
--------
Host (numpy): compute per-tree node depths from `parents`; process the
recurrence LEVEL-synchronously (all nodes at one depth are independent)
instead of the reference's 256 sequential steps.  Each core owns 16
trees and runs both directions (dt bottom-up, td top-down) as two
phases of one SPMD program.  Within a level, nodes are ordered grouped
by parent (parents in the previous level's order), which makes both
the td parent-gather and the dt children-segment-sum a STAIRCASE 0/1
matrix; gathers become small banded matmuls.

Device (Bass/Tile): per level
  sumHT  = prevH^T-gather via matmul (feature-major, feeds gates lhsT)
  sumC   = prevC row-major gather via matmul
  gates  = X^T @ W4x + sumH^T @ W4h   (W4 = [i,o,f,u] fused, 4H=1024)
  i,o,f  = sigmoid(gates[0:768]); u = tanh(gates[768:1024])   (ACT)
  c      = i*u + f*sumC;  h = o*tanh(c)                        (DVE)
All matmuls run in float32r (full fp32 storage, reduced-precision PE
multiply, 1 cycle/row).  PSUM accumulation groups with partial column
coverage are opened by a "zeroing matmul" (zeros lhsT, full width,
start=True) so untouched elements are exact zeros.

Schedules are data-dependent: level sizes are padded to the max over
the 8 cores (rounded to 128) so a single program serves all cores.
Host post-pass unpermutes the level-ordered outputs into [L, B, 2H].
"""

import os
import numpy as np

L, B, D, H = 256, 128, 256, 256
NCORES = 8
TPC = B // NCORES  # trees per core
H4 = 4 * H  # fused gate width (i,o,f,u)

_TRACE = os.environ.get("BITREE_TRACE", "0") == "1"
LAST_EXEC_NS = None

_CACHE = {}


def _install_ntff_shim():
    """Register the NTFF profile hook so trace=True works under axon."""
    import sys
    import types

    if "antenv.axon_hooks" in sys.modules:
        return
    hook_box = [None]
    mod = types.ModuleType("antenv.axon_hooks")
    mod.set_axon_ntff_profile_hook = lambda h: hook_box.__setitem__(0, h)
    mod.get_axon_ntff_profile_hook = lambda: hook_box[0]
    import antenv

    antenv.axon_hooks = mod
    sys.modules["antenv.axon_hooks"] = mod
    from trn_agent_boot.trn_boot import _ntff_profile_via_ctypes

    hook = _ntff_profile_via_ctypes("/opt/axon/libaxon_pjrt.so")
    if hook is not None:
        mod.set_axon_ntff_profile_hook(hook)


def _ceil128(x):
    return ((int(x) + 127) // 128) * 128


def _balance_trees(depth, NLV):
    """Assign trees to cores minimizing sum_d ceil128(max_core level_size)."""
    nd = np.zeros((NLV, B), np.int64)
    for b in range(B):
        cnt = np.bincount(depth[:, b], minlength=NLV)
        nd[:, b] = cnt
    def cost_of(p):
        # primary: padded chunk count; secondary: raw imbalance (plateau guide)
        return (int(((np.max(p, axis=1) + 127) // 128).sum()),
                int(np.max(p, axis=1).sum()))

    best_cost, best_cores = None, None
    for seed in range(3):
        rng = np.random.default_rng(seed)
        peak = nd.max(axis=0)
        orderb = np.argsort(-peak, kind="stable")
        cores = [[] for _ in range(NCORES)]
        prof = np.zeros((NLV, NCORES), np.int64)
        for b in orderb:
            bestj, bi = None, -1
            for c in range(NCORES):
                if len(cores[c]) >= TPC:
                    continue
                p = prof.copy()
                p[:, c] += nd[:, b]
                j = cost_of(p) + (int(p[:, c].sum()),)
                if bestj is None or j < bestj:
                    bestj, bi = j, c
            cores[bi].append(int(b))
            prof[:, bi] += nd[:, b]
        cur = cost_of(prof)
        for _ in range(20000):
            c1, c2 = rng.integers(0, NCORES, 2)
            if c1 == c2:
                continue
            i1, i2 = rng.integers(0, TPC, 2)
            b1, b2 = cores[c1][i1], cores[c2][i2]
            p = prof.copy()
            p[:, c1] += nd[:, b2] - nd[:, b1]
            p[:, c2] += nd[:, b1] - nd[:, b2]
            j = cost_of(p)
            if j <= cur:
                cur = j
                prof = p
                cores[c1][i1], cores[c2][i2] = b2, b1
        if best_cost is None or cur < best_cost:
            best_cost, best_cores = cur, [list(c) for c in cores]
    cores = best_cores
    tree_core = np.zeros(B, np.int64)
    for c in range(NCORES):
        cores[c].sort()
        for b in cores[c]:
            tree_core[b] = c
    return tree_core, [list(c) for c in cores]


def _build_schedule(parents):
    """Level schedule + gather-matrix band metadata, uniform across cores."""
    par = np.asarray(parents, dtype=np.int64)  # [L, B], par[0,:]=L
    depth = np.zeros((L, B), np.int64)
    bar = np.arange(B)
    for i in range(1, L):
        depth[i] = depth[par[i], bar] + 1
    DMAX = int(depth.max())
    NLV = DMAX + 1
    tree_core, core_trees = _balance_trees(depth, NLV)

    # per (level, core): ordered list of (tree, node); parent-grouped order
    order = [[[] for _ in range(NCORES)] for _ in range(NLV)]
    pos = np.full((L, B), -1, np.int64)
    for b in range(B):
        core = int(tree_core[b])
        kids = [[] for _ in range(L)]
        for i in range(1, L):
            kids[par[i, b]].append(i)
        cur = [0]
        d = 0
        while cur:
            od = order[d][core]
            for n in cur:
                pos[n, b] = len(od)
                od.append((b, n))
            nxt = []
            for n in cur:
                nxt.extend(kids[n])
            cur = nxt
            d += 1

    n_real = np.zeros((NLV, NCORES), np.int64)
    for d in range(NLV):
        for c in range(NCORES):
            n_real[d, c] = len(order[d][c])
    NPAD = [_ceil128(n_real[d].max()) for d in range(NLV)]

    # processing sequence: (phase, level, prev_level or None)
    # dt and td are independent chains — interleave them step-wise so each
    # chain's level-boundary dependency stall is covered by the other's work.
    dt_steps = [("dt", d, d + 1 if d < DMAX else None)
                for d in range(DMAX, -1, -1)]
    td_steps = [("td", d, d - 1 if d > 0 else None)
                for d in range(0, DMAX + 1)]
    steps = []
    for a, b_ in zip(dt_steps, td_steps):
        steps.append(a)
        steps.append(b_)

    # gather matrices: for step (phase, dl, pl): GT [m=NPAD[pl], n=NPAD[dl]]
    #   dt: GT[j, r] = 1 iff parent(order[pl][j]) == order[dl][r]
    #   td: GT[p, j] = 1 iff parent(order[dl][j]) == order[pl][p]
    # Build per-core col indices once (parent positions).
    def _gt_entries(phase, dl, pl, core):
        """row_idx, col_idx arrays of the 1-entries for this core."""
        if phase == "dt":
            ent = order[pl][core]  # children level
            rows = np.arange(len(ent), dtype=np.int64)
            cols = np.array([pos[par[n, b], b] for (b, n) in ent], dtype=np.int64)
        else:
            ent = order[dl][core]
            cols = np.arange(len(ent), dtype=np.int64)
            rows = np.array([pos[par[n, b], b] for (b, n) in ent], dtype=np.int64)
        return rows, cols

    # Band metadata per gather step: used chunks, 128-aligned col spans,
    # per-128-col-block windows (win1) and per-512-col-block windows (win2).
    gmeta = {}  # (phase, dl) -> dict
    for phase, dl, pl in steps:
        if pl is None:
            continue
        m, n = NPAD[pl], NPAD[dl]
        mch = m // 128
        # per-chunk tight col ranges, unioned over cores
        clo = np.full(mch, n, np.int64)
        chi = np.full(mch, -1, np.int64)
        ents = []
        for core in range(NCORES):
            rows, cols = _gt_entries(phase, dl, pl, core)
            ents.append((rows, cols))
            if len(rows):
                ch = rows // 128
                np.minimum.at(clo, ch, cols)
                np.maximum.at(chi, ch, cols)
        used = [c for c in range(mch) if chi[c] >= 0]
        span = {}
        for c in used:
            a = 128 * (clo[c] // 128)
            bnd = min(n, _ceil128(chi[c] + 1))
            span[c] = (int(a), int(bnd))
        # tight per-(chunk, 512-col-block) column ranges for gather2 matmuls
        nbk = (n + 511) // 512
        cliplo = np.full((mch, nbk), n, np.int64)
        cliphi = np.full((mch, nbk), -1, np.int64)
        for rows, cols in ents:
            if len(rows):
                key = (rows // 128, cols // 512)
                np.minimum.at(cliplo, key, cols)
                np.maximum.at(cliphi, key, cols + 1)
        clip = {(int(c), int(nb)): (int(cliplo[c, nb]) // 2 * 2,
                                    min(n, 512 * int(nb) + 512,
                                        (int(cliphi[c, nb]) + 1) // 2 * 2))
                for c in range(mch) for nb in range(nbk) if cliphi[c, nb] >= 0}
        # win1: per 128-col block i -> list of chunks with a 1 in that block
        # win2: per 512-col block nb -> same
        nch = n // 128
        w1 = [[] for _ in range(nch)]
        nb_cnt = (n + 511) // 512
        w2 = [[] for _ in range(nb_cnt)]
        touch1 = np.zeros((mch, nch), bool)
        for rows, cols in ents:
            if len(rows):
                touch1[rows // 128, cols // 128] = True
        for c in used:
            for i in range(nch):
                if touch1[c, i]:
                    w1[i].append(c)
            for nb in range(nb_cnt):
                if touch1[c, 4 * nb : min(nch, 4 * nb + 4)].any():
                    w2[nb].append(c)
        gmeta[(phase, dl)] = dict(used=used, span=span, w1=w1, w2=w2, m=m, n=n, clip=clip)
        # store entries for data build
        gmeta[(phase, dl)]["ents"] = ents

    # layout offsets
    xt_off, acc = [], 0
    for d in range(NLV):
        xt_off.append(acc)
        acc += 2 * NPAD[d]
    XTW = acc

    gt_off = {}  # (phase, dl, chunk) -> col offset in gt tensor
    acc = 0
    gt_level_off = {}
    for phase, dl, pl in steps:
        if pl is None:
            continue
        gm = gmeta[(phase, dl)]
        gt_level_off[(phase, dl)] = acc
        for c in gm["used"]:
            a, bnd = gm["span"][c]
            gt_off[(phase, dl, c)] = acc
            acc += bnd - a
    GTW = max(acc, 128)

    out_off = {}
    acc = 0
    for phase, dl, pl in steps:
        out_off[(phase, dl)] = acc
        acc += NPAD[dl]
    OUTR = acc

    return dict(
        DMAX=DMAX, NLV=NLV, order=order, pos=pos, n_real=n_real, NPAD=NPAD,
        steps=steps, gmeta=gmeta, xt_off=xt_off, XTW=XTW, gt_off=gt_off,
        gt_level_off=gt_level_off, GTW=GTW, out_off=out_off, OUTR=OUTR,
        tree_core=tree_core,
    )


def _build_core_inputs(sched, inputs_np, weights):
    """Per-core numpy arrays: xt [128, XTW], gt [128, GTW], shared w4t."""
    NPAD, xt_off, XTW = sched["NPAD"], sched["xt_off"], sched["XTW"]
    GTW, gt_off = sched["GTW"], sched["gt_off"]
    order, steps, gmeta = sched["order"], sched["steps"], sched["gmeta"]
    NLV = sched["NLV"]

    xts, gts = [], []
    for core in range(NCORES):
        xt = np.zeros((128, XTW), np.float32)
        for d in range(NLV):
            ent = order[d][core]
            if ent:
                bs = np.array([b for b, n in ent])
                ns = np.array([n for b, n in ent])
                xl = inputs_np[ns, bs, :]  # [n_d, 256]
                xlT = xl.T  # [256, n_d]
                o = xt_off[d]
                xt[:, o : o + len(ent)] = xlT[:128]
                xt[:, o + NPAD[d] : o + NPAD[d] + len(ent)] = xlT[128:]
        xts.append(xt)

        gt = np.zeros((128, GTW), np.float32)
        for phase, dl, pl in steps:
            if pl is None:
                continue
            gm = gmeta[(phase, dl)]
            rows, cols = gm["ents"][core]
            if not len(rows):
                continue
            for c in gm["used"]:
                a, bnd = gm["span"][c]
                msk = (rows // 128) == c
                if not msk.any():
                    continue
                r = rows[msk] - 128 * c
                cc = cols[msk] - a
                o = gt_off[(phase, dl, c)]
                gt[r, o + cc] = 1.0
        gts.append(gt)

    # fused weights: per direction, rows ordered [i, o, f, u], transposed.
    # layout [128, 8192]: dir (dt=0, td=1) at 4096*dir; x-part chunks k at
    # [doff + k*1024, +1024), h-part at [doff + 2048 + k*1024, +1024).
    w4t = np.zeros((128, 8192), np.float32)
    for di, pre in enumerate(("dt", "td")):
        ioux, iouh = weights[f"{pre}_ioux_w"], weights[f"{pre}_iouh_w"]
        fx, fh = weights[f"{pre}_fx_w"], weights[f"{pre}_fh_w"]
        wx = np.concatenate([ioux[0:256], fx, ioux[256:512], ioux[512:768]], 0)
        wh = np.concatenate([iouh[0:256], fh, iouh[256:512], iouh[512:768]], 0)
        for k in range(2):
            w4t[:, di * 4096 + k * 1024 : di * 4096 + (k + 1) * 1024] = \
                wx.T[k * 128 : (k + 1) * 128]
            w4t[:, di * 4096 + 2048 + k * 1024 : di * 4096 + 2048 + (k + 1) * 1024] = \
                wh.T[k * 128 : (k + 1) * 128]
    return xts, gts, w4t


def _build_program(sched):
    from contextlib import ExitStack

    import concourse.tile as tile
    from concourse import bacc, mybir

    f32 = mybir.dt.float32
    f32r = mybir.dt.float32r
    SIG = mybir.ActivationFunctionType.Sigmoid
    TANH = mybir.ActivationFunctionType.Tanh

    NPAD, xt_off = sched["NPAD"], sched["xt_off"]
    XTW, GTW, OUTR = sched["XTW"], sched["GTW"], sched["OUTR"]
    steps, gmeta = sched["steps"], sched["gmeta"]
    gt_off, gt_level_off = sched["gt_off"], sched["gt_level_off"]
    out_off = sched["out_off"]

    nc = bacc.Bacc("TRN2", target_bir_lowering=False, debug=False,
                   num_devices=NCORES)

    xt_ap = nc.dram_tensor("xt", [128, XTW], f32r, kind="ExternalInput").ap()
    gt_ap = nc.dram_tensor("gt", [128, GTW], f32r, kind="ExternalInput").ap()
    w4_ap = nc.dram_tensor("w4t", [128, 8192], f32r, kind="ExternalInput").ap()
    z_ap = nc.dram_tensor("zeros", [128, 128], f32r, kind="ExternalInput").ap()
    oc_ap = nc.dram_tensor("out_c", [OUTR, 256], f32, kind="ExternalOutput").ap()
    oh_ap = nc.dram_tensor("out_h", [OUTR, 256], f32, kind="ExternalOutput").ap()

    with tile.TileContext(nc) as tc:
        with ExitStack() as ctx:
            const = ctx.enter_context(tc.tile_pool(name="const", bufs=1))
            xt_pool = ctx.enter_context(tc.tile_pool(name="xt", bufs=3))
            gt_pool = ctx.enter_context(tc.tile_pool(name="gt", bufs=3))
            cpool = ctx.enter_context(tc.tile_pool(name="stc", bufs=2))
            hpool = ctx.enter_context(tc.tile_pool(name="sth", bufs=2))
            sht_pool = ctx.enter_context(tc.tile_pool(name="sht", bufs=3))
            tmp = ctx.enter_context(tc.tile_pool(name="tmp", bufs=3))
            ps2s = ctx.enter_context(tc.tile_pool(name="ps2", bufs=2, space="PSUM"))
            ps2 = {"dt": ps2s, "td": ps2s}
            ps1s = ctx.enter_context(tc.tile_pool(name="ps1", bufs=2, space="PSUM"))
            ps1 = {"dt": ps1s, "td": ps1s}
            psg = ctx.enter_context(tc.tile_pool(name="psg", bufs=2, space="PSUM"))

            # weights as 4 lazily-loaded tiles (dir x {x-part, h-part}) so the
            # first gate matmul only waits on its own 1MB slice.
            w4_tiles = {}

            def w4_tile(di, part):  # part: 0=x, 1=h
                key = (di, part)
                if key not in w4_tiles:
                    t = const.tile([128, 2048], f32r, name=f"w4_{di}_{part}")
                    nc.sync.dma_start(
                        t[:], w4_ap[:, di * 4096 + part * 2048 :
                                    di * 4096 + part * 2048 + 2048])
                    w4_tiles[key] = t
                return w4_tiles[key]

            zer_t = const.tile([128, 128], f32r)
            nc.sync.dma_start(zer_t[:], z_ap[:])

            prev = {"dt": (None, None), "td": (None, None)}
            for phase, dl, pl in steps:
                di = 0 if phase == "dt" else 1
                prev_c, prev_h = prev[phase]
                n = NPAD[dl]
                nch = n // 128
                xo = xt_off[dl]

                xt_t = xt_pool.tile([128, 2 * n], f32r, tag="xt")
                nc.sync.dma_start(xt_t[:], xt_ap[:, xo : xo + 2 * n])

                cur_c = cpool.tile([128, nch * 256], f32r, tag=f"stc_{phase}")
                cur_h = hpool.tile([128, nch * 256], f32r, tag=f"sth_{phase}")

                has_prev = pl is not None
                if has_prev:
                    gm = gmeta[(phase, dl)]
                    lvl_go = gt_level_off[(phase, dl)]
                    lvl_w = sum(gm["span"][c][1] - gm["span"][c][0]
                                for c in gm["used"])
                    gt_t = gt_pool.tile([128, max(lvl_w, 128)], f32r, tag="gt")
                    if lvl_w:
                        nc.sync.dma_start(gt_t[:, :lvl_w],
                                          gt_ap[:, lvl_go : lvl_go + lvl_w])

                    # gather2: sumHT [2][128, n] feature-major
                    sht = sht_pool.tile([128, 2 * n], f32r, tag="sht")
                    for j in range(2):
                        for nb in range((n + 511) // 512):
                            nb0 = 512 * nb
                            wb = min(512, n - nb0)
                            mms = []
                            for c in gm["w2"][nb]:
                                a, bnd = gm["span"][c]
                                lo, hi = gm["clip"][(c, nb)]
                                if lo < hi:
                                    mms.append((c, a, lo, hi))
                            pst = ps2[phase].tile([128, wb], f32, tag="ps2")
                            nc.tensor.matmul(pst[:], zer_t[:],
                                             xt_t[:, 0:wb],
                                             start=True, stop=not mms,
                                             skip_group_check=True)
                            for mi, (c, a, lo, hi) in enumerate(mms):
                                go = gt_off[(phase, dl, c)] - lvl_go
                                nc.tensor.matmul(
                                    pst[:, lo - nb0 : hi - nb0],
                                    prev_h[:, c * 256 + j * 128 :
                                           c * 256 + j * 128 + 128],
                                    gt_t[:, go + lo - a : go + hi - a],
                                    start=False, stop=(mi == len(mms) - 1),
                                    skip_group_check=True)
                            nc.vector.tensor_copy(
                                sht[:, j * n + nb0 : j * n + nb0 + wb], pst[:])

                for i in range(nch):
                    io = i * 128
                    # gather1: sumC for this row-chunk
                    ps_c = None
                    if has_prev:
                        gm = gmeta[(phase, dl)]
                        lvl_go = gt_level_off[(phase, dl)]
                        ps_c = ps1[phase].tile([128, 256], f32, tag="ps1")
                        w1 = gm["w1"][i]
                        if not w1:
                            nc.tensor.matmul(ps_c[:], zer_t[:],
                                             prev_c[:, 0:256],
                                             start=True, stop=True,
                                             skip_group_check=True)
                        # every real matmul fully covers [128, 256] (spans are
                        # 128-aligned), so the first one opens the group.
                        for ci, c in enumerate(w1):
                            a, bnd = gm["span"][c]
                            go = gt_off[(phase, dl, c)] - lvl_go
                            nc.tensor.matmul(
                                ps_c[:], gt_t[:, go + io - a : go + io - a + 128],
                                prev_c[:, c * 256 : c * 256 + 256],
                                start=(ci == 0), stop=(ci == len(w1) - 1),
                                skip_group_check=True)

                    # gates: [128, 1024] = X^T W4x (+ sumH^T W4h)
                    w4x = w4_tile(di, 0)
                    ps_g = psg.tile([128, 1024], f32, tag="psg")
                    for half in range(2):
                        ho = half * 512
                        for k in range(2):
                            nc.tensor.matmul(
                                ps_g[:, ho : ho + 512],
                                xt_t[:, k * n + io : k * n + io + 128],
                                w4x[:, k * 1024 + ho : k * 1024 + ho + 512],
                                start=(k == 0), stop=(k == 1 and not has_prev),
                                skip_group_check=True)
                        if has_prev:
                            w4h = w4_tile(di, 1)
                            for k in range(2):
                                nc.tensor.matmul(
                                    ps_g[:, ho : ho + 512],
                                    sht[:, k * n + io : k * n + io + 128],
                                    w4h[:, k * 1024 + ho : k * 1024 + ho + 512],
                                    start=False, stop=(k == 1),
                                    skip_group_check=True)

                    # cell math (gate order [i, f, o, u]; o off the c-path)
                    s_if = tmp.tile([128, 512], f32, tag="sif")
                    nc.scalar.activation(s_if[:], ps_g[:, 0:512], SIG)
                    u_t = tmp.tile([128, 256], f32, tag="ut")
                    nc.scalar.activation(u_t[:], ps_g[:, 768:1024], TANH)

                    ccol = cur_c[:, i * 256 : i * 256 + 256]
                    hcol = cur_h[:, i * 256 : i * 256 + 256]
                    if has_prev:
                        t1 = tmp.tile([128, 256], f32, tag="t1")
                        nc.vector.tensor_mul(t1[:], s_if[:, 0:256], u_t[:])
                        t2 = tmp.tile([128, 256], f32, tag="t2")
                        nc.vector.tensor_mul(t2[:], s_if[:, 256:512], ps_c[:])
                        nc.vector.tensor_add(ccol, t1[:], t2[:])
                    else:
                        nc.vector.tensor_mul(ccol, s_if[:, 0:256], u_t[:])
                    tc_t = tmp.tile([128, 256], f32, tag="tct")
                    nc.scalar.activation(tc_t[:], ccol.bitcast(f32), TANH)
                    s_o = tmp.tile([128, 256], f32, tag="so")
                    nc.scalar.activation(s_o[:], ps_g[:, 512:768], SIG)
                    nc.vector.tensor_mul(hcol, s_o[:], tc_t[:])

                ro = out_off[(phase, dl)]
                oc_v = oc_ap[ro : ro + n, :].rearrange(
                    "(c p) f -> p c f", p=128)
                oh_v = oh_ap[ro : ro + n, :].rearrange(
                    "(c p) f -> p c f", p=128)
                nc.sync.dma_start(
                    oc_v, cur_c[:].bitcast(f32).rearrange("p (c f) -> p c f", f=256))
                nc.sync.dma_start(
                    oh_v, cur_h[:].bitcast(f32).rearrange("p (c f) -> p c f", f=256))

                prev[phase] = (cur_c, cur_h)

    nc.compile()
    return nc


def kernel(**inputs):
    global LAST_EXEC_NS
    inp = {k: np.asarray(v) for k, v in inputs.items()}
    x = inp["inputs"].astype(np.float32)
    parents = inp["parents"]

    for pre in ("dt", "td"):
        for nm in ("ioux", "iouh", "fx", "fh"):
            if np.any(inp[f"{pre}_{nm}_b"] != 0):
                raise NotImplementedError("nonzero biases not supported")

    key = parents.tobytes()
    if key not in _CACHE:
        sched = _build_schedule(parents)
        prog = _build_program(sched)
        _CACHE[key] = (sched, prog)
    sched, prog = _CACHE[key]

    xts, gts, w4t = _build_core_inputs(sched, x, inp)
    zeros = np.zeros((128, 128), np.float32)
    in_maps = [
        {"xt": xts[c], "gt": gts[c], "w4t": w4t, "zeros": zeros}
        for c in range(NCORES)
    ]

    from concourse.bass_utils import run_bass_kernel_spmd

    if _TRACE:
        _install_ntff_shim()
        res = run_bass_kernel_spmd(prog, in_maps, list(range(NCORES)), trace=True)
        LAST_EXEC_NS = res.exec_time_ns
        print(f"HW exec time: {res.exec_time_ns} ns")
    else:
        res = run_bass_kernel_spmd(prog, in_maps, list(range(NCORES)))

    cells = np.zeros((L, B, 2 * H), np.float32)
    hiddens = np.zeros((L, B, 2 * H), np.float32)
    order, n_real = sched["order"], sched["n_real"]
    out_off, NLV = sched["out_off"], sched["NLV"]
    for core in range(NCORES):
        oc = res.results[core]["out_c"]
        oh = res.results[core]["out_h"]
        for phase, sl in (("dt", slice(0, H)), ("td", slice(H, 2 * H))):
            for d in range(NLV):
                ent = order[d][core]
                if not ent:
                    continue
                o = out_off[(phase, d)]
                bs = np.array([b for b, n_ in ent])
                ns = np.array([n_ for b, n_ in ent])
                cells[ns, bs, sl] = oc[o : o + len(ent)]
                hiddens[ns, bs, sl] = oh[o : o + len(ent)]
    return cells, hiddens


# revision 25
# speedup vs baseline: 1.0272x; 1.0122x over previous
"""BiTreeLSTM Trainium2 kernel (8 NeuronCores, SPMD, batch-sharded).

Strategy
--------
Host (numpy): compute per-tree node depths from `parents`; process the
recurrence LEVEL-synchronously (all nodes at one depth are independent)
instead of the reference's 256 sequential steps.  Each core owns 16
trees and runs both directions (dt bottom-up, td top-down) as two
phases of one SPMD program.  Within a level, nodes are ordered grouped
by parent (parents in the previous level's order), which makes both
the td parent-gather and the dt children-segment-sum a STAIRCASE 0/1
matrix; gathers become small banded matmuls.

Device (Bass/Tile): per level
  sumHT  = prevH^T-gather via matmul (feature-major, feeds gates lhsT)
  sumC   = prevC row-major gather via matmul
  gates  = X^T @ W4x + sumH^T @ W4h   (W4 = [i,o,f,u] fused, 4H=1024)
  i,o,f  = sigmoid(gates[0:768]); u = tanh(gates[768:1024])   (ACT)
  c      = i*u + f*sumC;  h = o*tanh(c)                        (DVE)
All matmuls run in float32r (full fp32 storage, reduced-precision PE
multiply, 1 cycle/row).  PSUM accumulation groups with partial column
coverage are opened by a "zeroing matmul" (zeros lhsT, full width,
start=True) so untouched elements are exact zeros.

Schedules are data-dependent: level sizes are padded to the max over
the 8 cores (rounded to 128) so a single program serves all cores.
Host post-pass unpermutes the level-ordered outputs into [L, B, 2H].
"""

import os
import numpy as np

L, B, D, H = 256, 128, 256, 256
NCORES = 8
TPC = B // NCORES  # trees per core
H4 = 4 * H  # fused gate width (i,o,f,u)

_TRACE = os.environ.get("BITREE_TRACE", "0") == "1"
LAST_EXEC_NS = None

_CACHE = {}


def _install_ntff_shim():
    """Register the NTFF profile hook so trace=True works under axon."""
    import sys
    import types

    if "antenv.axon_hooks" in sys.modules:
        return
    hook_box = [None]
    mod = types.ModuleType("antenv.axon_hooks")
    mod.set_axon_ntff_profile_hook = lambda h: hook_box.__setitem__(0, h)
    mod.get_axon_ntff_profile_hook = lambda: hook_box[0]
    import antenv

    antenv.axon_hooks = mod
    sys.modules["antenv.axon_hooks"] = mod
    from trn_agent_boot.trn_boot import _ntff_profile_via_ctypes

    hook = _ntff_profile_via_ctypes("/opt/axon/libaxon_pjrt.so")
    if hook is not None:
        mod.set_axon_ntff_profile_hook(hook)


def _ceil128(x):
    return ((int(x) + 127) // 128) * 128


def _balance_trees(depth, NLV):
    """Assign trees to cores minimizing sum_d ceil128(max_core level_size)."""
    nd = np.zeros((NLV, B), np.int64)
    for b in range(B):
        cnt = np.bincount(depth[:, b], minlength=NLV)
        nd[:, b] = cnt
    def cost_of(p):
        # primary: padded chunk count; secondary: raw imbalance (plateau guide)
        return (int(((np.max(p, axis=1) + 127) // 128).sum()),
                int(np.max(p, axis=1).sum()))

    best_cost, best_cores = None, None
    for seed in range(3):
        rng = np.random.default_rng(seed)
        peak = nd.max(axis=0)
        orderb = np.argsort(-peak, kind="stable")
        cores = [[] for _ in range(NCORES)]
        prof = np.zeros((NLV, NCORES), np.int64)
        for b in orderb:
            bestj, bi = None, -1
            for c in range(NCORES):
                if len(cores[c]) >= TPC:
                    continue
                p = prof.copy()
                p[:, c] += nd[:, b]
                j = cost_of(p) + (int(p[:, c].sum()),)
                if bestj is None or j < bestj:
                    bestj, bi = j, c
            cores[bi].append(int(b))
            prof[:, bi] += nd[:, b]
        cur = cost_of(prof)
        for _ in range(20000):
            c1, c2 = rng.integers(0, NCORES, 2)
            if c1 == c2:
                continue
            i1, i2 = rng.integers(0, TPC, 2)
            b1, b2 = cores[c1][i1], cores[c2][i2]
            p = prof.copy()
            p[:, c1] += nd[:, b2] - nd[:, b1]
            p[:, c2] += nd[:, b1] - nd[:, b2]
            j = cost_of(p)
            if j <= cur:
                cur = j
                prof = p
                cores[c1][i1], cores[c2][i2] = b2, b1
        if best_cost is None or cur < best_cost:
            best_cost, best_cores = cur, [list(c) for c in cores]
    cores = best_cores
    tree_core = np.zeros(B, np.int64)
    for c in range(NCORES):
        cores[c].sort()
        for b in cores[c]:
            tree_core[b] = c
    return tree_core, [list(c) for c in cores]


def _build_schedule(parents):
    """Level schedule + gather-matrix band metadata, uniform across cores."""
    par = np.asarray(parents, dtype=np.int64)  # [L, B], par[0,:]=L
    depth = np.zeros((L, B), np.int64)
    bar = np.arange(B)
    for i in range(1, L):
        depth[i] = depth[par[i], bar] + 1
    DMAX = int(depth.max())
    NLV = DMAX + 1
    tree_core, core_trees = _balance_trees(depth, NLV)

    # per (level, core): ordered list of (tree, node); parent-grouped order
    order = [[[] for _ in range(NCORES)] for _ in range(NLV)]
    pos = np.full((L, B), -1, np.int64)
    for b in range(B):
        core = int(tree_core[b])
        kids = [[] for _ in range(L)]
        for i in range(1, L):
            kids[par[i, b]].append(i)
        cur = [0]
        d = 0
        while cur:
            od = order[d][core]
            for n in cur:
                pos[n, b] = len(od)
                od.append((b, n))
            nxt = []
            for n in cur:
                nxt.extend(kids[n])
            cur = nxt
            d += 1

    n_real = np.zeros((NLV, NCORES), np.int64)
    for d in range(NLV):
        for c in range(NCORES):
            n_real[d, c] = len(order[d][c])
    NPAD = [_ceil128(n_real[d].max()) for d in range(NLV)]

    # processing sequence: (phase, level, prev_level or None)
    # dt and td are independent chains — interleave them step-wise so each
    # chain's level-boundary dependency stall is covered by the other's work.
    dt_steps = [("dt", d, d + 1 if d < DMAX else None)
                for d in range(DMAX, -1, -1)]
    td_steps = [("td", d, d - 1 if d > 0 else None)
                for d in range(0, DMAX + 1)]
    steps = []
    for a, b_ in zip(dt_steps, td_steps):
        steps.append(a)
        steps.append(b_)

    # gather matrices: for step (phase, dl, pl): GT [m=NPAD[pl], n=NPAD[dl]]
    #   dt: GT[j, r] = 1 iff parent(order[pl][j]) == order[dl][r]
    #   td: GT[p, j] = 1 iff parent(order[dl][j]) == order[pl][p]
    # Build per-core col indices once (parent positions).
    def _gt_entries(phase, dl, pl, core):
        """row_idx, col_idx arrays of the 1-entries for this core."""
        if phase == "dt":
            ent = order[pl][core]  # children level
            rows = np.arange(len(ent), dtype=np.int64)
            cols = np.array([pos[par[n, b], b] for (b, n) in ent], dtype=np.int64)
        else:
            ent = order[dl][core]
            cols = np.arange(len(ent), dtype=np.int64)
            rows = np.array([pos[par[n, b], b] for (b, n) in ent], dtype=np.int64)
        return rows, cols

    # Band metadata per gather step: used chunks, 128-aligned col spans,
    # per-128-col-block windows (win1) and per-512-col-block windows (win2).
    gmeta = {}  # (phase, dl) -> dict
    for phase, dl, pl in steps:
        if pl is None:
            continue
        m, n = NPAD[pl], NPAD[dl]
        mch = m // 128
        # per-chunk tight col ranges, unioned over cores
        clo = np.full(mch, n, np.int64)
        chi = np.full(mch, -1, np.int64)
        ents = []
        for core in range(NCORES):
            rows, cols = _gt_entries(phase, dl, pl, core)
            ents.append((rows, cols))
            if len(rows):
                ch = rows // 128
                np.minimum.at(clo, ch, cols)
                np.maximum.at(chi, ch, cols)
        used = [c for c in range(mch) if chi[c] >= 0]
        span = {}
        for c in used:
            a = 128 * (clo[c] // 128)
            bnd = min(n, _ceil128(chi[c] + 1))
            span[c] = (int(a), int(bnd))
        # tight per-(chunk, 512-col-block) column ranges for gather2 matmuls
        nbk = (n + 511) // 512
        cliplo = np.full((mch, nbk), n, np.int64)
        cliphi = np.full((mch, nbk), -1, np.int64)
        for rows, cols in ents:
            if len(rows):
                key = (rows // 128, cols // 512)
                np.minimum.at(cliplo, key, cols)
                np.maximum.at(cliphi, key, cols + 1)
        clip = {(int(c), int(nb)): (int(cliplo[c, nb]) // 2 * 2,
                                    min(n, 512 * int(nb) + 512,
                                        (int(cliphi[c, nb]) + 1) // 2 * 2))
                for c in range(mch) for nb in range(nbk) if cliphi[c, nb] >= 0}
        # win1: per 128-col block i -> list of chunks with a 1 in that block
        # win2: per 512-col block nb -> same
        nch = n // 128
        w1 = [[] for _ in range(nch)]
        nb_cnt = (n + 511) // 512
        w2 = [[] for _ in range(nb_cnt)]
        touch1 = np.zeros((mch, nch), bool)
        for rows, cols in ents:
            if len(rows):
                touch1[rows // 128, cols // 128] = True
        for c in used:
            for i in range(nch):
                if touch1[c, i]:
                    w1[i].append(c)
            for nb in range(nb_cnt):
                if touch1[c, 4 * nb : min(nch, 4 * nb + 4)].any():
                    w2[nb].append(c)
        gmeta[(phase, dl)] = dict(used=used, span=span, w1=w1, w2=w2, m=m, n=n, clip=clip)
        # store entries for data build
        gmeta[(phase, dl)]["ents"] = ents

    # layout offsets
    xt_off, acc = [], 0
    for d in range(NLV):
        xt_off.append(acc)
        acc += 2 * NPAD[d]
    XTW = acc

    gt_off = {}  # (phase, dl, chunk) -> col offset in gt tensor
    acc = 0
    gt_level_off = {}
    for phase, dl, pl in steps:
        if pl is None:
            continue
        gm = gmeta[(phase, dl)]
        gt_level_off[(phase, dl)] = acc
        for c in gm["used"]:
            a, bnd = gm["span"][c]
            gt_off[(phase, dl, c)] = acc
            acc += bnd - a
    GTW = max(acc, 128)

    out_off = {}
    acc = 0
    for phase, dl, pl in steps:
        out_off[(phase, dl)] = acc
        acc += NPAD[dl]
    OUTR = acc

    return dict(
        DMAX=DMAX, NLV=NLV, order=order, pos=pos, n_real=n_real, NPAD=NPAD,
        steps=steps, gmeta=gmeta, xt_off=xt_off, XTW=XTW, gt_off=gt_off,
        gt_level_off=gt_level_off, GTW=GTW, out_off=out_off, OUTR=OUTR,
        tree_core=tree_core,
    )


def _build_core_inputs(sched, inputs_np, weights):
    """Per-core numpy arrays: xt [128, XTW], gt [128, GTW], shared w4t."""
    NPAD, xt_off, XTW = sched["NPAD"], sched["xt_off"], sched["XTW"]
    GTW, gt_off = sched["GTW"], sched["gt_off"]
    order, steps, gmeta = sched["order"], sched["steps"], sched["gmeta"]
    NLV = sched["NLV"]

    xts, gts = [], []
    for core in range(NCORES):
        xt = np.zeros((128, XTW), np.float32)
        for d in range(NLV):
            ent = order[d][core]
            if ent:
                bs = np.array([b for b, n in ent])
                ns = np.array([n for b, n in ent])
                xl = inputs_np[ns, bs, :]  # [n_d, 256]
                xlT = xl.T  # [256, n_d]
                o = xt_off[d]
                xt[:, o : o + len(ent)] = xlT[:128]
                xt[:, o + NPAD[d] : o + NPAD[d] + len(ent)] = xlT[128:]
        xts.append(xt)

        gt = np.zeros((128, GTW), np.float32)
        for phase, dl, pl in steps:
            if pl is None:
                continue
            gm = gmeta[(phase, dl)]
            rows, cols = gm["ents"][core]
            if not len(rows):
                continue
            for c in gm["used"]:
                a, bnd = gm["span"][c]
                msk = (rows // 128) == c
                if not msk.any():
                    continue
                r = rows[msk] - 128 * c
                cc = cols[msk] - a
                o = gt_off[(phase, dl, c)]
                gt[r, o + cc] = 1.0
        gts.append(gt)

    # fused weights: per direction, rows ordered [i, o, f, u], transposed.
    # layout [128, 8192]: dir (dt=0, td=1) at 4096*dir; x-part chunks k at
    # [doff + k*1024, +1024), h-part at [doff + 2048 + k*1024, +1024).
    w4t = np.zeros((128, 8192), np.float32)
    for di, pre in enumerate(("dt", "td")):
        ioux, iouh = weights[f"{pre}_ioux_w"], weights[f"{pre}_iouh_w"]
        fx, fh = weights[f"{pre}_fx_w"], weights[f"{pre}_fh_w"]
        wx = np.concatenate([ioux[0:256], fx, ioux[256:512], ioux[512:768]], 0)
        wh = np.concatenate([iouh[0:256], fh, iouh[256:512], iouh[512:768]], 0)
        for k in range(2):
            w4t[:, di * 4096 + k * 1024 : di * 4096 + (k + 1) * 1024] = \
                wx.T[k * 128 : (k + 1) * 128]
            w4t[:, di * 4096 + 2048 + k * 1024 : di * 4096 + 2048 + (k + 1) * 1024] = \
                wh.T[k * 128 : (k + 1) * 128]
    return xts, gts, w4t


def _build_program(sched):
    from contextlib import ExitStack

    import concourse.tile as tile
    from concourse import bacc, mybir

    f32 = mybir.dt.float32
    f32r = mybir.dt.float32r
    SIG = mybir.ActivationFunctionType.Sigmoid
    TANH = mybir.ActivationFunctionType.Tanh

    NPAD, xt_off = sched["NPAD"], sched["xt_off"]
    XTW, GTW, OUTR = sched["XTW"], sched["GTW"], sched["OUTR"]
    steps, gmeta = sched["steps"], sched["gmeta"]
    gt_off, gt_level_off = sched["gt_off"], sched["gt_level_off"]
    out_off = sched["out_off"]

    nc = bacc.Bacc("TRN2", target_bir_lowering=False, debug=False,
                   num_devices=NCORES)

    xt_ap = nc.dram_tensor("xt", [128, XTW], f32r, kind="ExternalInput").ap()
    gt_ap = nc.dram_tensor("gt", [128, GTW], f32r, kind="ExternalInput").ap()
    w4_ap = nc.dram_tensor("w4t", [128, 8192], f32r, kind="ExternalInput").ap()
    z_ap = nc.dram_tensor("zeros", [128, 128], f32r, kind="ExternalInput").ap()
    oc_ap = nc.dram_tensor("out_c", [OUTR, 256], f32, kind="ExternalOutput").ap()
    oh_ap = nc.dram_tensor("out_h", [OUTR, 256], f32, kind="ExternalOutput").ap()

    with tile.TileContext(nc) as tc:
        with ExitStack() as ctx:
            const = ctx.enter_context(tc.tile_pool(name="const", bufs=1))
            xt_pool = ctx.enter_context(tc.tile_pool(name="xt", bufs=3))
            gt_pool = ctx.enter_context(tc.tile_pool(name="gt", bufs=3))
            cpool = ctx.enter_context(tc.tile_pool(name="stc", bufs=2))
            hpool = ctx.enter_context(tc.tile_pool(name="sth", bufs=2))
            sht_pool = ctx.enter_context(tc.tile_pool(name="sht", bufs=3))
            tmp = ctx.enter_context(tc.tile_pool(name="tmp", bufs=3))
            ps2s = ctx.enter_context(tc.tile_pool(name="ps2", bufs=2, space="PSUM"))
            ps2 = {"dt": ps2s, "td": ps2s}
            ps1s = ctx.enter_context(tc.tile_pool(name="ps1", bufs=2, space="PSUM"))
            ps1 = {"dt": ps1s, "td": ps1s}
            psg = ctx.enter_context(tc.tile_pool(name="psg", bufs=2, space="PSUM"))

            # weights as 4 lazily-loaded tiles (dir x {x-part, h-part}) so the
            # first gate matmul only waits on its own 1MB slice.
            w4_tiles = {}

            def w4_tile(di, part):  # part: 0=x, 1=h
                key = (di, part)
                if key not in w4_tiles:
                    t = const.tile([128, 2048], f32r, name=f"w4_{di}_{part}")
                    nc.sync.dma_start(
                        t[:], w4_ap[:, di * 4096 + part * 2048 :
                                    di * 4096 + part * 2048 + 2048])
                    w4_tiles[key] = t
                return w4_tiles[key]

            zer_t = const.tile([128, 128], f32r)
            nc.sync.dma_start(zer_t[:], z_ap[:])

            # HAM warmup: keep the PE busy during the initial DMA window so
            # the first real matmuls run at 2.4 GHz instead of 1.2.
            warm_ps = ps2["dt"].tile([128, 128], f32, tag="ps2", name="warm")
            for _ in range(36):
                nc.tensor.matmul(warm_ps[:], zer_t[:], zer_t[:],
                                 start=True, stop=True, skip_group_check=True)

            prev = {"dt": (None, None), "td": (None, None)}
            for phase, dl, pl in steps:
                di = 0 if phase == "dt" else 1
                prev_c, prev_h = prev[phase]
                n = NPAD[dl]
                nch = n // 128
                xo = xt_off[dl]

                xt_t = xt_pool.tile([128, 2 * n], f32r, tag="xt")
                nc.sync.dma_start(xt_t[:], xt_ap[:, xo : xo + 2 * n])

                cur_c = cpool.tile([128, nch * 256], f32r, tag=f"stc_{phase}")
                cur_h = hpool.tile([128, nch * 256], f32r, tag=f"sth_{phase}")

                has_prev = pl is not None
                if has_prev:
                    gm = gmeta[(phase, dl)]
                    lvl_go = gt_level_off[(phase, dl)]
                    lvl_w = sum(gm["span"][c][1] - gm["span"][c][0]
                                for c in gm["used"])
                    gt_t = gt_pool.tile([128, max(lvl_w, 128)], f32r, tag="gt")
                    if lvl_w:
                        nc.sync.dma_start(gt_t[:, :lvl_w],
                                          gt_ap[:, lvl_go : lvl_go + lvl_w])

                    # gather2: sumHT [2][128, n] feature-major
                    sht = sht_pool.tile([128, 2 * n], f32r, tag="sht")
                    for j in range(2):
                        for nb in range((n + 511) // 512):
                            nb0 = 512 * nb
                            wb = min(512, n - nb0)
                            mms = []
                            for c in gm["w2"][nb]:
                                a, bnd = gm["span"][c]
                                lo, hi = gm["clip"][(c, nb)]
                                if lo < hi:
                                    mms.append((c, a, lo, hi))
                            pst = ps2[phase].tile([128, wb], f32, tag="ps2")
                            nc.tensor.matmul(pst[:], zer_t[:],
                                             xt_t[:, 0:wb],
                                             start=True, stop=not mms,
                                             skip_group_check=True)
                            for mi, (c, a, lo, hi) in enumerate(mms):
                                go = gt_off[(phase, dl, c)] - lvl_go
                                nc.tensor.matmul(
                                    pst[:, lo - nb0 : hi - nb0],
                                    prev_h[:, c * 256 + j * 128 :
                                           c * 256 + j * 128 + 128],
                                    gt_t[:, go + lo - a : go + hi - a],
                                    start=False, stop=(mi == len(mms) - 1),
                                    skip_group_check=True)
                            nc.vector.tensor_copy(
                                sht[:, j * n + nb0 : j * n + nb0 + wb], pst[:])

                for i in range(nch):
                    io = i * 128
                    # gather1: sumC for this row-chunk
                    ps_c = None
                    if has_prev:
                        gm = gmeta[(phase, dl)]
                        lvl_go = gt_level_off[(phase, dl)]
                        ps_c = ps1[phase].tile([128, 256], f32, tag="ps1")
                        w1 = gm["w1"][i]
                        if not w1:
                            nc.tensor.matmul(ps_c[:], zer_t[:],
                                             prev_c[:, 0:256],
                                             start=True, stop=True,
                                             skip_group_check=True)
                        # every real matmul fully covers [128, 256] (spans are
                        # 128-aligned), so the first one opens the group.
                        for ci, c in enumerate(w1):
                            a, bnd = gm["span"][c]
                            go = gt_off[(phase, dl, c)] - lvl_go
                            nc.tensor.matmul(
                                ps_c[:], gt_t[:, go + io - a : go + io - a + 128],
                                prev_c[:, c * 256 : c * 256 + 256],
                                start=(ci == 0), stop=(ci == len(w1) - 1),
                                skip_group_check=True)

                    # gates: [128, 1024] = X^T W4x (+ sumH^T W4h)
                    w4x = w4_tile(di, 0)
                    ps_g = psg.tile([128, 1024], f32, tag="psg")
                    for half in range(2):
                        ho = half * 512
                        for k in range(2):
                            nc.tensor.matmul(
                                ps_g[:, ho : ho + 512],
                                xt_t[:, k * n + io : k * n + io + 128],
                                w4x[:, k * 1024 + ho : k * 1024 + ho + 512],
                                start=(k == 0), stop=(k == 1 and not has_prev),
                                skip_group_check=True)
                        if has_prev:
                            w4h = w4_tile(di, 1)
                            for k in range(2):
                                nc.tensor.matmul(
                                    ps_g[:, ho : ho + 512],
                                    sht[:, k * n + io : k * n + io + 128],
                                    w4h[:, k * 1024 + ho : k * 1024 + ho + 512],
                                    start=False, stop=(k == 1),
                                    skip_group_check=True)

                    # cell math (gate order [i, f, o, u]; o off the c-path)
                    s_if = tmp.tile([128, 512], f32, tag="sif")
                    nc.scalar.activation(s_if[:], ps_g[:, 0:512], SIG)
                    u_t = tmp.tile([128, 256], f32, tag="ut")
                    nc.scalar.activation(u_t[:], ps_g[:, 768:1024], TANH)

                    ccol = cur_c[:, i * 256 : i * 256 + 256]
                    hcol = cur_h[:, i * 256 : i * 256 + 256]
                    if has_prev:
                        t1 = tmp.tile([128, 256], f32, tag="t1")
                        nc.vector.tensor_mul(t1[:], s_if[:, 0:256], u_t[:])
                        t2 = tmp.tile([128, 256], f32, tag="t2")
                        nc.vector.tensor_mul(t2[:], s_if[:, 256:512], ps_c[:])
                        nc.vector.tensor_add(ccol, t1[:], t2[:])
                    else:
                        nc.vector.tensor_mul(ccol, s_if[:, 0:256], u_t[:])
                    tc_t = tmp.tile([128, 256], f32, tag="tct")
                    nc.scalar.activation(tc_t[:], ccol.bitcast(f32), TANH)
                    s_o = tmp.tile([128, 256], f32, tag="so")
                    nc.scalar.activation(s_o[:], ps_g[:, 512:768], SIG)
                    nc.vector.tensor_mul(hcol, s_o[:], tc_t[:])

                ro = out_off[(phase, dl)]
                oc_v = oc_ap[ro : ro + n, :].rearrange(
                    "(c p) f -> p c f", p=128)
                oh_v = oh_ap[ro : ro + n, :].rearrange(
                    "(c p) f -> p c f", p=128)
                nc.sync.dma_start(
                    oc_v, cur_c[:].bitcast(f32).rearrange("p (c f) -> p c f", f=256))
                nc.sync.dma_start(
                    oh_v, cur_h[:].bitcast(f32).rearrange("p (c f) -> p c f", f=256))

                prev[phase] = (cur_c, cur_h)

    nc.compile()
    return nc


def kernel(**inputs):
    global LAST_EXEC_NS
    inp = {k: np.asarray(v) for k, v in inputs.items()}
    x = inp["inputs"].astype(np.float32)
    parents = inp["parents"]

    for pre in ("dt", "td"):
        for nm in ("ioux", "iouh", "fx", "fh"):
            if np.any(inp[f"{pre}_{nm}_b"] != 0):
                raise NotImplementedError("nonzero biases not supported")

    key = parents.tobytes()
    if key not in _CACHE:
        sched = _build_schedule(parents)
        prog = _build_program(sched)
        _CACHE[key] = (sched, prog)
    sched, prog = _CACHE[key]

    xts, gts, w4t = _build_core_inputs(sched, x, inp)
    zeros = np.zeros((128, 128), np.float32)
    in_maps = [
        {"xt": xts[c], "gt": gts[c], "w4t": w4t, "zeros": zeros}
        for c in range(NCORES)
    ]

    from concourse.bass_utils import run_bass_kernel_spmd

    if _TRACE:
        _install_ntff_shim()
        res = run_bass_kernel_spmd(prog, in_maps, list(range(NCORES)), trace=True)
        LAST_EXEC_NS = res.exec_time_ns
        print(f"HW exec time: {res.exec_time_ns} ns")
    else:
        res = run_bass_kernel_spmd(prog, in_maps, list(range(NCORES)))

    cells = np.zeros((L, B, 2 * H), np.float32)
    hiddens = np.zeros((L, B, 2 * H), np.float32)
    order, n_real = sched["order"], sched["n_real"]
    out_off, NLV = sched["out_off"], sched["NLV"]
    for core in range(NCORES):
        oc = res.results[core]["out_c"]
        oh = res.results[core]["out_h"]
        for phase, sl in (("dt", slice(0, H)), ("td", slice(H, 2 * H))):
            for d in range(NLV):
                ent = order[d][core]
                if not ent:
                    continue
                o = out_off[(phase, d)]
                bs = np.array([b for b, n_ in ent])
                ns = np.array([n_ for b, n_ in ent])
                cells[ns, bs, sl] = oc[o : o + len(ent)]
                hiddens[ns, bs, sl] = oh[o : o + len(ent)]
    return cells, hiddens
